# revision 22
# baseline (speedup 1.0000x reference)
"""Head-sharded Blenderbot MHA forward, one NeuronCore per 2 heads (v6).

Sharding: D (=16 heads) split across 8 cores -> 128 out-channels (2 heads)
of Q/K/V per core; out_lin is row-parallel (each core computes a full
[B*S, D] partial from its 128 ctx channels); the host sums the 8 fp16
partials (the "all-reduce") and adds out_b. No device-to-device traffic.

v6 changes vs v5 (195.8us): rebalance engines around the ACT exp wall.
 - ACT runs ONLY the 128 exp instructions during attention (the machine
   floor: 131072 elem/partition @ 1.2GHz = 109us + per-op init). All
   evacuations move to DVE.
 - ctx matmul is FLIPPED: stationary = e-tile column block [keys 128,
   q 128], moving = V [keys 128, DH+1], out = [q 128, DH+1] in PSUM.
   Cost model charges out-free-size (65) instead of moving 1024 per
   sk: 131K -> 67K PE cycles. The denominator rides as V's ones
   column and lands PER-PARTITION (per query), so softmax
   normalization becomes reciprocal [128,8] + one broadcast
   tensor_tensor multiply fused with the evacuation - the v5
   DRAM-round-trip broadcast machinery is gone.
 - ctx comes out [q, dh]-oriented; PE transposes (bf16 identity, 1
   cyc/row) restore ctxT [chan, q] for the out-projection stationary.
 - PSUM: sA/sB score ping-pong (2+2 banks), ctx accumulator (2), P
   (proj chunks / V+ctx transposes / outproj, 2). ctx tile is zeroed
   by two [128,512] matmuls (stationary zeros) so the per-qb
   sub-range accumulation never relies on partial-bank
   start_tensor_calc semantics.
 - PE p-state: only the first matmul after an idle gap pays the mid
   p-state; the work-queue keeps PE saturated with proj/outproj/
   transpose filler so scores stay full-speed.
"""

import functools
from collections import deque
from contextlib import ExitStack

import ml_dtypes
import numpy as np

import concourse.bass as bass
import concourse.tile as tile
from concourse import mybir
from concourse.bass_utils import run_bass_kernel_spmd

B, S, D, H, DH = 2, 2048, 1024, 16, 64
N_CORES = 8
DPC = D // N_CORES        # 128 = 2 heads
BS = B * S
NQC = S // 1024           # 2
NST = S // 128            # 16
NKT = D // 128            # 8
QB = 8                    # 128-query blocks per 1024-query pass

F32 = mybir.dt.float32
F32R = mybir.dt.float32r
F16 = mybir.dt.float16
BF16 = mybir.dt.bfloat16
Act = mybir.ActivationFunctionType
Alu = mybir.AluOpType

FEED_PER_ITER = 2
DRAIN_PER_ITER = 3
PRIO_SCORES = 18
PRIO_OP = 15
PRIO_NORM = 10
MARKS = []


def _mark(nc, label):
    MARKS.append((int(nc.next_id()), label))


def _split_sync_commands(nc, max_waits=1, max_updates=8):
    for fn in nc.m.functions:
        for bb in fn.blocks:
            new_insts = []
            changed = False
            for inst in bb.instructions:
                si = getattr(inst, "sync_info", None)
                if si is not None:
                    waits = list(si.on_wait or [])
                    if len(waits) > max_waits:
                        for w in waits[:-max_waits]:
                            new_insts.append(mybir.InstNoOp(
                                name=nc.get_next_instruction_name(),
                                ins=[], outs=[], engine=inst.engine,
                                sync_info=mybir.SyncInfo(on_wait=[w], on_update=[]),
                            ))
                        si.on_wait = waits[-max_waits:]
                        changed = True
                    updates = list(si.on_update or [])
                    if len(updates) > max_updates:
                        si.on_update = updates[:max_updates]
                        new_insts.append(inst)
                        new_insts.append(mybir.InstNoOp(
                            name=nc.get_next_instruction_name(),
                            ins=[], outs=[], engine=inst.engine,
                            sync_info=mybir.SyncInfo(
                                on_wait=[], on_update=updates[max_updates:]),
                        ))
                        changed = True
                        continue
                new_insts.append(inst)
            if changed:
                bb.instructions = new_insts


def _free_reshape(ap, dims):
    """Reinterpret a [P, N] AP's free dim as nested dims (row-major)."""
    new = [list(ap.ap[0])]
    stride = ap.ap[-1][0]
    total = 1
    for d in dims:
        total *= d
    assert total == ap.ap[-1][1], (dims, ap.ap)
    rem = total
    for d in dims:
        rem //= d
        new.append([stride * rem, d])
    return bass.AP(tensor=ap.tensor, offset=ap.offset, ap=new)


def _bcast_free(ap, n):
    """[P, M] AP -> [P, M, n] with a 0-stride broadcast last dim."""
    return bass.AP(tensor=ap.tensor, offset=ap.offset,
                   ap=[list(p) for p in ap.ap] + [[0, n]])


F8 = mybir.dt.float8e4


@functools.lru_cache(maxsize=1)
def _build():
    nc = bass.Bass()
    # qtc[0] = fp8(x^T), qtc[1] = fp8(x^T - qtc[0]): 3-term DoubleRow
    # projection (x8 w8 + x8 wr8 + xr8 w8) carries bf16-level accuracy at
    # 0.75x the bf16 PE cost (2x contraction per pass, 0.5 cyc/row).
    qtc_d = nc.dram_tensor("qtc", [2, D, BS], F8, kind="ExternalInput")
    wq_d = nc.dram_tensor("wq", [2, 128, NKT * DPC], F8, kind="ExternalInput")
    wk_d = nc.dram_tensor("wk", [2, 128, NKT * DPC], F8, kind="ExternalInput")
    wv_d = nc.dram_tensor("wv", [2, 128, NKT * DPC], F8, kind="ExternalInput")
    bq_d = nc.dram_tensor("bq", [DPC, 1], F32, kind="ExternalInput")
    bk_d = nc.dram_tensor("bk", [DPC, 1], F32, kind="ExternalInput")
    bv_d = nc.dram_tensor("bv", [DPC, 1], F32, kind="ExternalInput")
    wo_d = nc.dram_tensor("wo", [DPC, D], BF16, kind="ExternalInput")
    out_d = nc.dram_tensor("out_part", [BS, D], F16, kind="ExternalOutput")
    ident_d = nc.inline_tensor(
        np.eye(128, dtype=np.float32).astype(ml_dtypes.bfloat16), "ident")

    with tile.TileContext(nc) as tc, ExitStack() as ctx:
        consts = ctx.enter_context(tc.tile_pool(name="consts", bufs=1))
        qt_pool = ctx.enter_context(tc.tile_pool(name="qt", bufs=2))
        projp = ctx.enter_context(tc.tile_pool(name="proj", bufs=2))
        vtp = ctx.enter_context(tc.tile_pool(name="vtp", bufs=2))
        vpool = ctx.enter_context(tc.tile_pool(name="vpool", bufs=2))
        ctxp = ctx.enter_context(tc.tile_pool(name="ctxp", bufs=2))
        expp = ctx.enter_context(tc.tile_pool(name="expp", bufs=18))
        normp = ctx.enter_context(tc.tile_pool(name="normp", bufs=2))
        outp = ctx.enter_context(tc.tile_pool(name="outp", bufs=6))
        psp = ctx.enter_context(tc.tile_pool(name="psp", bufs=1, space="PSUM"))

        def ps_tile(shape, tag):
            return psp.tile(shape, F32, tag=tag, name="ps_" + tag)

        # ---- constants ------------------------------------------------------
        def _wpair(dram, sb, eng):
            # [2, 128, NKT*DPC] dram -> [128, 2, NKT, DPC] sbuf in one DMA
            src = dram[:, :, :]
            n = NKT * DPC
            eng.dma_start(out=sb, in_=bass.AP(
                tensor=src.tensor, offset=src.offset,
                ap=[[n, 128], [128 * n, 2], [DPC, NKT], [1, DPC]]))

        wq_sb = consts.tile([128, 2, NKT, DPC], F8, tag="wq")
        wk_sb = consts.tile([128, 2, NKT, DPC], F8, tag="wk")
        wv_sb = consts.tile([128, 2, NKT, DPC], F8, tag="wv")
        wo_sb = consts.tile([128, D], BF16, tag="wo")
        bq_sb = consts.tile([128, 1], F32, tag="bq")
        bk_sb = consts.tile([128, 1], F32, tag="bk")
        bv_sb = consts.tile([128, 1], F32, tag="bv")
        ident_sb = consts.tile([128, 128], BF16, tag="ident")
        zw_sb = consts.tile([128, 512], BF16, tag="zw")
        nc.vector.memset(zw_sb, 0.0)
        sixt_sb = consts.tile([128, 1], F32, tag="sixt")
        nc.vector.memset(sixt_sb, 1.0 / 16.0)
        zero_sb = consts.tile([128, 1], F32, tag="zero")
        nc.vector.memset(zero_sb, 0.0)

        # Everything on the SP queue: HWDGE is one serial device, so a single
        # queue gives exact arrival ordering (w -> qt h0 pairs -> biases ->
        # the rest).
        def load_consts_head():
            _wpair(wk_d, wk_sb, nc.sync)
            _wpair(wq_d, wq_sb, nc.sync)

        def load_consts_rest():
            nc.sync.dma_start(out=bk_sb, in_=bk_d[:, :])
            nc.sync.dma_start(out=bq_sb, in_=bq_d[:, :])
            nc.sync.dma_start(out=bv_sb, in_=bv_d[:, :])
            _wpair(wv_d, wv_sb, nc.scalar)
            nc.scalar.dma_start(out=wo_sb, in_=wo_d[:, :])
            nc.scalar.dma_start(out=ident_sb, in_=ident_d[:, :])

        state = {}

        # ------------------- work queue machinery ---------------------------
        FWQ = deque()          # groups: [items_deque, needs]
        PROVIDED = set()
        pending = deque()      # (thunk, needs_marker_or_None)
        ACTIVE = [None]
        cur_items = [None]

        def fw(fn, provides=None):
            assert cur_items[0] is not None, "fw() outside a group"
            cur_items[0].append((fn, provides))

        def group(needs=None):
            from contextlib import contextmanager

            @contextmanager
            def _cm():
                items = deque()
                FWQ.append([items, needs])
                prev = cur_items[0]
                cur_items[0] = items
                try:
                    yield
                finally:
                    cur_items[0] = prev
            return _cm()

        def _run_item(g):
            fn, prov = g[0].popleft()
            fn()
            if prov is not None:
                if isinstance(prov, list):
                    PROVIDED.update(prov)
                else:
                    PROVIDED.add(prov)
            if not g[0]:
                if ACTIVE[0] is g:
                    ACTIVE[0] = None
                if g in FWQ:
                    FWQ.remove(g)

        def feed_one():
            g = ACTIVE[0]
            if g is not None:
                if g[1] is None or g[1] in PROVIDED:
                    _run_item(g)
                    return True
                return False
            for i, cand in enumerate(FWQ):
                if i >= 16:
                    break
                if cand[1] is None or cand[1] in PROVIDED:
                    ACTIVE[0] = cand
                    _run_item(cand)
                    return True
            return False

        def feed(n):
            for _ in range(n):
                if not feed_one():
                    return

        def feed_until(marker):
            spins = 0
            while marker not in PROVIDED:
                if not feed_one():
                    try_drain(4)
                    spins += 1
                    assert spins < 2000, f"feed_until({marker}) stuck"

        def try_drain(n):
            done = 0
            while pending and done < n:
                fn, needs = pending[0]
                if needs is not None and needs not in PROVIDED:
                    return
                pending.popleft()
                fn()
                done += 1

        def drain_all():
            while pending:
                fn, needs = pending[0]
                if needs is not None and needs not in PROVIDED:
                    feed_until(needs)
                pending.popleft()
                fn()

        # ------------------------- loads ------------------------------------
        # qtc SBUF layout: [128, 2(term), NKT, S] fp8. One DMA per
        # (kt-pair, column-half) moves both terms: pair j lands complete so
        # DoubleRow k-steps can stream behind the transfers.
        def _qt_dma(b, qt_sb, t, k0, nk, h, eng):
            """One 3D transfer: term t, kt rows [k0, k0+nk), column-half h."""
            src = qtc_d[:, :, :]
            eng.dma_start(
                out=qt_sb[:, t, k0:k0 + nk, h * 1024:(h + 1) * 1024],
                in_=bass.AP(
                    tensor=src.tensor,
                    offset=src.offset + t * D * BS + (k0 * 128) * BS
                    + b * S + h * 1024,
                    ap=[[BS, 128], [128 * BS, nk], [1, 1024]]))

        def load_qt_stream_h0(b):
            """b0 column-half 0, pair-by-pair (both terms): the warmup
            projections stream behind the transfers."""
            qt_sb = qt_pool.tile([128, 2, NKT, S], F8, tag="qt")
            state[b, "qt"] = qt_sb
            for j in range(NKT // 2):
                for t in range(2):
                    _qt_dma(b, qt_sb, t, 2 * j, 2, 0, nc.sync)

        def load_qt_h1(b):
            qt_sb = state[b, "qt"]
            for t in range(2):
                for kp in range(2):
                    _qt_dma(b, qt_sb, t, 4 * kp, 4, 1, nc.sync)

        def load_qt_bulk(b, eng):
            qt_sb = qt_pool.tile([128, 2, NKT, S], F8, tag="qt")
            state[b, "qt"] = qt_sb
            for h in range(2):
                for t in range(2):
                    for kp in range(2):
                        _qt_dma(b, qt_sb, t, 4 * kp, 4, h, eng)

        # ------------------------- projections ------------------------------
        def alloc_proj(b):
            state[b, "QT"] = projp.tile([128, S], BF16, tag="QT", name="QT")
            state[b, "KT"] = projp.tile([128, S], BF16, tag="KT", name="KT")
            state[b, "VT"] = vtp.tile([128, S], BF16, tag="VT", name="VT")

        def alloc_v(b):
            V = vpool.tile([128, NST, 2, DH + 1], BF16, tag="V", name="V")
            nc.vector.memset(V[:, :, :, DH:DH + 1], 1.0)
            state[b, "V"] = V

        DR = mybir.MatmulPerfMode.DoubleRow

        def proj_mm_j(ps, b, which, pc, j):
            """kt-pair j of a projection chunk: 3 DoubleRow terms x 2 halves."""
            _mark(nc, f"proj_mm[{b}]{which}{pc}")
            qt_sb = state[b, "qt"]
            w_sb = {"q": wq_sb, "k": wk_sb, "v": wv_sb}[which]
            sl = slice(2 * j, 2 * j + 2)
            for hh in range(2):
                cs = slice(pc * 1024 + hh * 512, pc * 1024 + (hh + 1) * 512)
                for t, (wt, xt) in enumerate(
                        ((0, 0), (1, 0), (0, 1))):  # (w8,x8),(wr8,x8),(w8,xr8)
                    nc.tensor.matmul(
                        ps[:, hh * 512:(hh + 1) * 512],
                        w_sb[:, wt, sl, :], qt_sb[:, xt, sl, cs],
                        start=(j == 0 and t == 0), stop=(j == 3 and t == 2),
                        perf_mode=DR, skip_group_check=True)

        def proj_evac(ps, b, which, pc):
            _mark(nc, f"proj_ev[{b}]{which}{pc}")
            w_b = {"q": bq_sb, "k": bk_sb, "v": bv_sb}[which]
            dst = state[b, {"q": "QT", "k": "KT", "v": "VT"}[which]]
            with tc.high_priority(offset=12):
                nc.vector.tensor_scalar(
                    out=dst[:, pc * 1024:(pc + 1) * 1024], in0=ps,
                    scalar1=sixt_sb, scalar2=w_b, op0=Alu.mult, op1=Alu.add)

        def fw_proj_chunk(b, which, pc, provides=None):
            holder = {}

            def mm(j):
                if "ps" not in holder:
                    holder["ps"] = ps_tile([128, 1024], "P")
                proj_mm_j(holder["ps"], b, which, pc, j)

            with group():
                for j in range(NKT // 2):
                    fw(lambda j=j: mm(j))
                fw(lambda: proj_evac(holder["ps"], b, which, pc),
                   provides=provides)

        # ------------------------- V transpose ------------------------------
        def tr_quad(ps, b, st0):
            _mark(nc, f"tr[{b}]")
            VT = state[b, "VT"]
            psb = ps.bitcast(BF16)
            for i in range(4):
                nc.tensor.transpose(
                    psb[:, (st0 % 8 + i) * 128:(st0 % 8 + i + 1) * 128],
                    VT[:, (st0 + i) * 128:(st0 + i + 1) * 128], ident_sb)

        def tr_copy8(ps, b, st0):
            _mark(nc, f"trc[{b}]")
            V = state[b, "V"]
            dst = V[:, st0:st0 + 8, :, 0:DH]
            psb = ps.bitcast(BF16)
            nc.vector.tensor_copy(dst, _free_reshape(psb[:, 0:1024], (8, 2, DH)))

        def fw_tr_group(b, st0):
            holder = {}

            def quad(st):
                if "ps" not in holder:
                    holder["ps"] = ps_tile([128, 1024], "P")
                tr_quad(holder["ps"], b, st)

            def cpy():
                tr_copy8(holder["ps"], b, st0)

            with group():
                fw(lambda: quad(st0))
                fw(lambda: quad(st0 + 4))
                fw(cpy, provides=("trg", b, st0))

        # ------------------------- attention --------------------------------
        def alloc_attn(b):
            state[b, "ctxT"] = ctxp.tile([128, S], BF16, tag="ctxT", name="ctxT")

        PASS_ORDER = [(0, 0, 0), (0, 0, 1), (0, 1, 0), (0, 1, 1),
                      (1, 0, 0), (1, 0, 1), (1, 1, 0), (1, 1, 1)]

        def attention_pass(b, qc, u):
            QT, KT, V = state[b, "QT"], state[b, "KT"], state[b, "V"]
            tags = ("sA", "sB")
            pss = {}
            holder = {}
            pidx = PASS_ORDER.index((b, qc, u))

            def zero_ctx():
                _mark(nc, f"zctx[{b}]{qc}{u}")
                cps = psp.tile([128, QB, 128], F32, tag="ctx", name="ps_ctx")
                holder["c"] = cps
                flat = bass.AP(tensor=cps.tensor, offset=cps.offset,
                               ap=[list(cps.ap[0]), [1, 1024]])
                for hh in range(2):
                    nc.tensor.matmul(
                        flat[:, hh * 512:(hh + 1) * 512], zw_sb[:, 0:128],
                        zw_sb, start=True, stop=False, skip_group_check=True)

            def scores(sk):
                _mark(nc, f"scores[{b}]{qc}{u}")
                ps = ps_tile([128, 1024], tags[sk % 2])
                pss[sk] = ps
                with tc.high_priority(offset=PRIO_SCORES):
                    for hh in range(2):
                        nc.tensor.matmul(
                            ps[:, hh * 512:(hh + 1) * 512],
                            KT[u * DH:(u + 1) * DH, sk * 128:(sk + 1) * 128],
                            QT[u * DH:(u + 1) * DH,
                               qc * 1024 + hh * 512:qc * 1024 + (hh + 1) * 512],
                            start=True, stop=True)

            def ctx_mms(sk, e):
                _mark(nc, f"ctx[{b}]{qc}{u}")
                cps = holder["c"]
                for qb in range(QB):
                    nc.tensor.matmul(
                        cps[:, qb, 0:DH + 1],
                        e[:, qb * 128:(qb + 1) * 128],
                        V[:, sk, u, :],
                        start=False, stop=(sk == NST - 1),
                        skip_group_check=True)

            def norm_chain():
                _mark(nc, f"norm[{b}]{qc}{u}")
                cps = holder["c"]
                rep = normp.tile([128, QB], F32, tag="rep", name="rep")
                ctxn = normp.tile([128, QB, DH], BF16, tag="ctxn", name="ctxn")
                with tc.high_priority(offset=PRIO_NORM):
                    nc.vector.reciprocal(rep, cps[:, :, DH:DH + 1])
                    nc.vector.tensor_tensor(
                        out=ctxn, in0=cps[:, :, 0:DH],
                        in1=_bcast_free(rep[:, :], DH), op=Alu.mult)
                state[b, qc, u, "ctxn"] = ctxn
                PROVIDED.add(("ctxn", b, qc, u))

            if qc == 1:
                feed_until(("q1", b))
            pending.append((zero_ctx, None))
            scores(0)
            for sk in range(NST):
                ps = pss.pop(sk)
                _mark(nc, f"exp[{b}]{qc}{u}")
                # e-tile rotation safety: tile buffers recycle after `bufs`
                # allocations; readers (deferred ctx matmuls) must be EMITTED
                # before the buffer is reused. Force-advance when backlogged.
                spins = 0
                while len(pending) >= 14:
                    h = pending[0][1]
                    if h is not None and h not in PROVIDED:
                        feed_until(h)
                    try_drain(8)
                    spins += 1
                    assert spins < 200, "e backlog drain stuck"
                e = expp.tile([128, 1024], BF16, tag="exp", name="exp_t")
                nc.scalar.activation(e, ps, Act.Exp, bias=zero_sb, scale=1.0)
                pending.append(
                    (lambda sk=sk, e=e: ctx_mms(sk, e),
                     ("trg", b, 0 if sk < 8 else 8)))
                if sk + 1 < NST:
                    if sk + 1 == 8 and qc == 0:
                        feed_until(("k1", b))
                    scores(sk + 1)
                feed(FEED_PER_ITER)
                try_drain(DRAIN_PER_ITER)
            # normp (rep/ctxn) rotation safety: pass N's ctxn buffer is
            # reused at pass N+2 — its readers (the ctx-transpose group of
            # pass N) must be emitted first.
            norm_needs = ("trc",) + PASS_ORDER[pidx - 2] if pidx >= 2 else None
            pending.append((norm_chain, norm_needs))

        # ---------------- ctx transpose (PSUM -> ctxT) ----------------------
        def fw_tr_ctx(b, qc, u):
            holder = {}

            def quads(j):
                _mark(nc, f"ctr[{b}]{qc}{u}")
                if j == 0:
                    holder["ps"] = ps_tile([128, 1024], "P")
                ctxn = state[b, qc, u, "ctxn"]
                psb = holder["ps"].bitcast(BF16)
                for qb in range(4 * j, 4 * j + 4):
                    nc.tensor.transpose(
                        psb[u * DH:(u + 1) * DH, qb * 128:(qb + 1) * 128],
                        ctxn[:, qb, :], ident_sb)

            def ev2():
                _mark(nc, f"cev[{b}]{qc}{u}")
                psb = holder["ps"].bitcast(BF16)
                ctxT = state[b, "ctxT"]
                with tc.high_priority(offset=8):
                    nc.vector.tensor_copy(
                        ctxT[u * DH:(u + 1) * DH, qc * 1024:(qc + 1) * 1024],
                        psb[u * DH:(u + 1) * DH, 0:1024])

            provs = [("trc", b, qc, u)]
            if u == 1:
                provs.append(("ctxT", b, qc))
            with group(needs=("ctxn", b, qc, u)):
                fw(lambda: quads(0))
                fw(lambda: quads(1))
                fw(ev2, provides=provs)

        # ------------------------- out projection ---------------------------
        def outproj_mm(ps, b, st):
            _mark(nc, f"op_mm[{b}]")
            ctxT = state[b, "ctxT"]
            with tc.high_priority(offset=PRIO_OP):
                for oc in range(2):
                    nc.tensor.matmul(ps[:, oc * 512:(oc + 1) * 512],
                                     ctxT[:, st * 128:(st + 1) * 128],
                                     wo_sb[:, oc * 512:(oc + 1) * 512],
                                     start=True, stop=True)

        def outproj_evac(ps, o2, j):
            _mark(nc, "op_ev")
            nc.vector.tensor_copy(o2[:, j, :], ps)

        def outproj_store(o2, b, st0):
            _mark(nc, "op_st")
            dst = out_d[b * S + st0 * 128: b * S + (st0 + 2) * 128, :]
            nc.sync.dma_start(
                out=bass.AP(tensor=dst.tensor, offset=dst.offset,
                            ap=[[D, 128], [128 * D, 2], [1, D]]),
                in_=o2)

        def fw_outproj(b, sts):
            sts = list(sts)
            assert len(sts) % 2 == 0
            holder = {}

            def mm(st):
                holder["ps"] = ps_tile([128, 1024], "P")
                outproj_mm(holder["ps"], b, st)

            def ev(st, j):
                if j == 0:
                    holder["o2"] = outp.tile([128, 2, D], F16, tag="o",
                                             name="o2")
                outproj_evac(holder["ps"], holder["o2"], j)

            def stre(st0):
                outproj_store(holder["o2"], b, st0)

            for i, st in enumerate(sts):
                with group(needs=("ctxT", b, st // 8)):
                    fw(lambda st=st: mm(st))
                    fw(lambda st=st, j=i % 2: ev(st, j))
                    if i % 2 == 1:
                        fw(lambda st0=sts[i - 1]: stre(st0))

        # =========================== schedule ===============================
        load_consts_head()
        load_qt_stream_h0(0)
        load_consts_rest()
        load_qt_h1(0)
        alloc_proj(0)
        alloc_v(0)
        load_qt_bulk(1, nc.sync)
        # PE p-state ramp burn: the clock needs ~3us of continuous execution
        # to reach 2.4GHz; spend it on dummy matmuls while qt streams in so
        # the real warmup projections run at full speed.
        ramp_ps = psp.tile([128, QB, 128], F32, tag="ctx", name="ps_ramp")
        rflat = bass.AP(tensor=ramp_ps.tensor, offset=ramp_ps.offset,
                        ap=[list(ramp_ps.ap[0]), [1, 1024]])
        for i in range(60):
            nc.tensor.matmul(rflat[:, 0:64], zw_sb[:, 0:128], zw_sb[:, 0:64],
                             start=True, stop=True, skip_group_check=True)
        # k0/q0 interleaved on the two score tags: both consume the same qt
        # pairs as they stream in; evacs run on ACT (idle pre-attention) + DVE
        psK = ps_tile([128, 1024], "sA")
        psQ = ps_tile([128, 1024], "sB")
        for j in range(NKT // 2):
            proj_mm_j(psK, 0, "k", 0, j)
            proj_mm_j(psQ, 0, "q", 0, j)
        KT0, QT0 = state[0, "KT"], state[0, "QT"]
        nc.scalar.activation(KT0[:, 0:512], psK[:, 0:512],
                             Act.Identity, bias=bk_sb, scale=1.0 / 16.0)
        nc.vector.tensor_scalar(out=QT0[:, 0:512], in0=psQ[:, 0:512],
                                scalar1=sixt_sb, scalar2=bq_sb,
                                op0=Alu.mult, op1=Alu.add)
        nc.scalar.activation(KT0[:, 512:1024], psK[:, 512:1024],
                             Act.Identity, bias=bk_sb, scale=1.0 / 16.0)
        nc.vector.tensor_scalar(out=QT0[:, 512:1024], in0=psQ[:, 512:1024],
                                scalar1=sixt_sb, scalar2=bq_sb,
                                op0=Alu.mult, op1=Alu.add)
        alloc_attn(0)

        # b0 leftovers weave into attention(b0) qc0; b1's projections all run
        # inside the b0 window (qt is double-buffered) so window B's P tag is
        # free for the out-projections.
        fw_proj_chunk(0, "v", 0)
        fw_tr_group(0, 0)
        fw_proj_chunk(0, "k", 1, provides=("k1", 0))
        fw_proj_chunk(0, "q", 1, provides=("q1", 0))
        fw_proj_chunk(0, "v", 1)
        fw_tr_group(0, 8)

        def _alloc_b1():
            alloc_proj(1)
            alloc_v(1)
        with group():
            fw(_alloc_b1)
        fw_proj_chunk(1, "k", 0, provides=("k0", 1))
        fw_proj_chunk(1, "q", 0, provides=("q0", 1))

        attention_pass(0, 0, 0)
        fw_tr_ctx(0, 0, 0)
        fw_proj_chunk(1, "k", 1, provides=("k1", 1))
        fw_proj_chunk(1, "v", 0)
        fw_tr_group(1, 0)
        attention_pass(0, 0, 1)
        fw_tr_ctx(0, 0, 1)
        fw_proj_chunk(1, "v", 1)
        fw_tr_group(1, 8)
        fw_proj_chunk(1, "q", 1, provides=("q1", 1))
        fw_outproj(0, range(8))

        attention_pass(0, 1, 0)
        fw_tr_ctx(0, 1, 0)
        attention_pass(0, 1, 1)
        fw_tr_ctx(0, 1, 1)
        fw_outproj(0, range(8, NST))

        # ---- window B: attention(b1) + all outproj ------------------------
        feed_until(("k0", 1))
        feed_until(("q0", 1))
        alloc_attn(1)

        attention_pass(1, 0, 0)
        fw_tr_ctx(1, 0, 0)
        fw_outproj(1, range(8))
        attention_pass(1, 0, 1)
        fw_tr_ctx(1, 0, 1)

        attention_pass(1, 1, 0)
        fw_tr_ctx(1, 1, 0)
        attention_pass(1, 1, 1)
        fw_tr_ctx(1, 1, 1)
        drain_all()
        spins = 0
        while FWQ:
            if not feed_one():
                try_drain(4)
                spins += 1
                assert spins < 2000, "tail drain stuck"
        # pipelined tail: rotate three free 2-bank tags; each tile's evac is
        # split across DVE + ACT (both idle post-attention) to halve the
        # per-st chain latency; per-st stores
        tail_tags = ("sA", "sB", "P")
        for i, st in enumerate(range(8, NST)):
            ps = ps_tile([128, 1024], tail_tags[i % 3])
            outproj_mm(ps, 1, st)
            o_sb = outp.tile([128, 2, D], F16, tag="o", name="o2")
            nc.vector.tensor_copy(o_sb[:, 0, 0:512], ps[:, 0:512])
            nc.scalar.activation(o_sb[:, 0, 512:1024], ps[:, 512:1024],
                                 Act.Copy, bias=0.0, scale=1.0)
            nc.sync.dma_start(
                out=out_d[S + st * 128: S + (st + 1) * 128, :],
                in_=o_sb[:, 0, :])

    _split_sync_commands(nc)
    return nc


def _img(w):
    """[DPC, D] float array -> SBUF image [128, NKT*DPC] (same dtype)."""
    wt = w.T.reshape(NKT, 128, DPC).transpose(1, 0, 2)
    return np.ascontiguousarray(wt.reshape(128, NKT * DPC))


def _w_pair_img(w, sl, scale):
    """fp8 weight + residual images, stacked [2, 128, NKT*DPC].

    Values stored at 16x so the residual stays clear of e4m3 subnormals;
    the 1/16 is applied at evacuation time.
    """
    f8 = ml_dtypes.float8_e4m3
    w16 = (w[sl, :].astype(np.float32) * (16.0 * scale))
    w8 = w16.astype(f8)
    wr8 = (w16 - w8.astype(np.float32)).astype(f8)
    return np.ascontiguousarray(np.stack([_img(w8), _img(wr8)]))


def _prepare(query, q_w, q_b, k_w, k_b, v_w, v_b, out_w):
    bf = ml_dtypes.bfloat16
    f8 = ml_dtypes.float8_e4m3
    qt = np.ascontiguousarray(query.reshape(BS, D).T)  # [D, BS] f32
    qt8 = qt.astype(f8)
    qtr8 = (qt - qt8.astype(np.float32)).astype(f8)
    qtc = np.ascontiguousarray(np.stack([qt8, qtr8]))  # [2, D, BS]
    in_maps = []
    for c in range(N_CORES):
        sl = slice(c * DPC, (c + 1) * DPC)
        in_maps.append({
            "qtc": qtc,
            # softmax 1/sqrt(dh) folded into the Q weights/bias
            "wq": _w_pair_img(q_w, sl, 0.125),
            "wk": _w_pair_img(k_w, sl, 1.0),
            "wv": _w_pair_img(v_w, sl, 1.0),
            "bq": np.ascontiguousarray((q_b[sl] * 0.125).reshape(DPC, 1)),
            "bk": np.ascontiguousarray(k_b[sl].reshape(DPC, 1)),
            "bv": np.ascontiguousarray(v_b[sl].reshape(DPC, 1)),
            "wo": np.ascontiguousarray(out_w[:, sl].T).astype(bf),
        })
    return in_maps


def kernel(query, mask, q_w, q_b, k_w, k_b, v_w, v_b, out_w, out_b):
    query = np.asarray(query, dtype=np.float32)
    q_w = np.asarray(q_w, dtype=np.float32); q_b = np.asarray(q_b, dtype=np.float32)
    k_w = np.asarray(k_w, dtype=np.float32); k_b = np.asarray(k_b, dtype=np.float32)
    v_w = np.asarray(v_w, dtype=np.float32); v_b = np.asarray(v_b, dtype=np.float32)
    out_w = np.asarray(out_w, dtype=np.float32); out_b = np.asarray(out_b, dtype=np.float32)

    in_maps = _prepare(query, q_w, q_b, k_w, k_b, v_w, v_b, out_w)
    nc = _build()
    res = run_bass_kernel_spmd(nc, in_maps, core_ids=list(range(N_CORES)))
    out = np.zeros((BS, D), dtype=np.float32)
    for c in range(N_CORES):
        out += res.results[c]["out_part"]
    out += out_b[None, :]
    return out.reshape(B, S, D)


# revision 28
# speedup vs baseline: 1.0175x; 1.0175x over previous
"""Head-sharded Blenderbot MHA forward, one NeuronCore per 2 heads (v6).

Sharding: D (=16 heads) split across 8 cores -> 128 out-channels (2 heads)
of Q/K/V per core; out_lin is row-parallel (each core computes a full
[B*S, D] partial from its 128 ctx channels); the host sums the 8 fp16
partials (the "all-reduce") and adds out_b. No device-to-device traffic.

v6 changes vs v5 (195.8us): rebalance engines around the ACT exp wall.
 - ACT runs ONLY the 128 exp instructions during attention (the machine
   floor: 131072 elem/partition @ 1.2GHz = 109us + per-op init). All
   evacuations move to DVE.
 - ctx matmul is FLIPPED: stationary = e-tile column block [keys 128,
   q 128], moving = V [keys 128, DH+1], out = [q 128, DH+1] in PSUM.
   Cost model charges out-free-size (65) instead of moving 1024 per
   sk: 131K -> 67K PE cycles. The denominator rides as V's ones
   column and lands PER-PARTITION (per query), so softmax
   normalization becomes reciprocal [128,8] + one broadcast
   tensor_tensor multiply fused with the evacuation - the v5
   DRAM-round-trip broadcast machinery is gone.
 - ctx comes out [q, dh]-oriented; PE transposes (bf16 identity, 1
   cyc/row) restore ctxT [chan, q] for the out-projection stationary.
 - PSUM: sA/sB score ping-pong (2+2 banks), ctx accumulator (2), P
   (proj chunks / V+ctx transposes / outproj, 2). ctx tile is zeroed
   by two [128,512] matmuls (stationary zeros) so the per-qb
   sub-range accumulation never relies on partial-bank
   start_tensor_calc semantics.
 - PE p-state: only the first matmul after an idle gap pays the mid
   p-state; the work-queue keeps PE saturated with proj/outproj/
   transpose filler so scores stay full-speed.
"""

import functools
from collections import deque
from contextlib import ExitStack

import ml_dtypes
import numpy as np

import concourse.bass as bass
import concourse.tile as tile
from concourse import mybir
from concourse.bass_utils import run_bass_kernel_spmd

B, S, D, H, DH = 2, 2048, 1024, 16, 64
N_CORES = 8
DPC = D // N_CORES        # 128 = 2 heads
BS = B * S
NQC = S // 1024           # 2
NST = S // 128            # 16
NKT = D // 128            # 8
QB = 8                    # 128-query blocks per 1024-query pass

F32 = mybir.dt.float32
F32R = mybir.dt.float32r
F16 = mybir.dt.float16
BF16 = mybir.dt.bfloat16
Act = mybir.ActivationFunctionType
Alu = mybir.AluOpType

FEED_PER_ITER = 2
DRAIN_PER_ITER = 4
PRIO_SCORES = 18
PRIO_OP = 15
PRIO_NORM = 10
MARKS = []


def _mark(nc, label):
    MARKS.append((int(nc.next_id()), label))


def _split_sync_commands(nc, max_waits=1, max_updates=8):
    for fn in nc.m.functions:
        for bb in fn.blocks:
            new_insts = []
            changed = False
            for inst in bb.instructions:
                si = getattr(inst, "sync_info", None)
                if si is not None:
                    waits = list(si.on_wait or [])
                    if len(waits) > max_waits:
                        for w in waits[:-max_waits]:
                            new_insts.append(mybir.InstNoOp(
                                name=nc.get_next_instruction_name(),
                                ins=[], outs=[], engine=inst.engine,
                                sync_info=mybir.SyncInfo(on_wait=[w], on_update=[]),
                            ))
                        si.on_wait = waits[-max_waits:]
                        changed = True
                    updates = list(si.on_update or [])
                    if len(updates) > max_updates:
                        si.on_update = updates[:max_updates]
                        new_insts.append(inst)
                        new_insts.append(mybir.InstNoOp(
                            name=nc.get_next_instruction_name(),
                            ins=[], outs=[], engine=inst.engine,
                            sync_info=mybir.SyncInfo(
                                on_wait=[], on_update=updates[max_updates:]),
                        ))
                        changed = True
                        continue
                new_insts.append(inst)
            if changed:
                bb.instructions = new_insts


def _free_reshape(ap, dims):
    """Reinterpret a [P, N] AP's free dim as nested dims (row-major)."""
    new = [list(ap.ap[0])]
    stride = ap.ap[-1][0]
    total = 1
    for d in dims:
        total *= d
    assert total == ap.ap[-1][1], (dims, ap.ap)
    rem = total
    for d in dims:
        rem //= d
        new.append([stride * rem, d])
    return bass.AP(tensor=ap.tensor, offset=ap.offset, ap=new)


def _bcast_free(ap, n):
    """[P, M] AP -> [P, M, n] with a 0-stride broadcast last dim."""
    return bass.AP(tensor=ap.tensor, offset=ap.offset,
                   ap=[list(p) for p in ap.ap] + [[0, n]])


F8 = mybir.dt.float8e4


@functools.lru_cache(maxsize=1)
def _build():
    nc = bass.Bass()
    # qtc[0] = fp8(x^T), qtc[1] = fp8(x^T - qtc[0]): 3-term DoubleRow
    # projection (x8 w8 + x8 wr8 + xr8 w8) carries bf16-level accuracy at
    # 0.75x the bf16 PE cost (2x contraction per pass, 0.5 cyc/row).
    qtc_d = nc.dram_tensor("qtc", [2, D, BS], F8, kind="ExternalInput")
    wq_d = nc.dram_tensor("wq", [2, 128, NKT * DPC], F8, kind="ExternalInput")
    wk_d = nc.dram_tensor("wk", [2, 128, NKT * DPC], F8, kind="ExternalInput")
    wv_d = nc.dram_tensor("wv", [2, 128, NKT * DPC], F8, kind="ExternalInput")
    bq_d = nc.dram_tensor("bq", [DPC, 1], F32, kind="ExternalInput")
    bk_d = nc.dram_tensor("bk", [DPC, 1], F32, kind="ExternalInput")
    bv_d = nc.dram_tensor("bv", [DPC, 1], F32, kind="ExternalInput")
    wo_d = nc.dram_tensor("wo", [DPC, D], BF16, kind="ExternalInput")
    out_d = nc.dram_tensor("out_part", [BS, D], F16, kind="ExternalOutput")
    ident_d = nc.inline_tensor(
        np.eye(128, dtype=np.float32).astype(ml_dtypes.bfloat16), "ident")

    with tile.TileContext(nc) as tc, ExitStack() as ctx:
        consts = ctx.enter_context(tc.tile_pool(name="consts", bufs=1))
        qt_pool = ctx.enter_context(tc.tile_pool(name="qt", bufs=2))
        projp = ctx.enter_context(tc.tile_pool(name="proj", bufs=2))
        vtp = ctx.enter_context(tc.tile_pool(name="vtp", bufs=2))
        vpool = ctx.enter_context(tc.tile_pool(name="vpool", bufs=2))
        ctxp = ctx.enter_context(tc.tile_pool(name="ctxp", bufs=2))
        expp = ctx.enter_context(tc.tile_pool(name="expp", bufs=18))
        normp = ctx.enter_context(tc.tile_pool(name="normp", bufs=2))
        outp = ctx.enter_context(tc.tile_pool(name="outp", bufs=6))
        psp = ctx.enter_context(tc.tile_pool(name="psp", bufs=1, space="PSUM"))

        def ps_tile(shape, tag):
            return psp.tile(shape, F32, tag=tag, name="ps_" + tag)

        # ---- constants ------------------------------------------------------
        def _wpair(dram, sb, eng):
            # [2, 128, NKT*DPC] dram -> [128, 2, NKT, DPC] sbuf in one DMA
            src = dram[:, :, :]
            n = NKT * DPC
            eng.dma_start(out=sb, in_=bass.AP(
                tensor=src.tensor, offset=src.offset,
                ap=[[n, 128], [128 * n, 2], [DPC, NKT], [1, DPC]]))

        wq_sb = consts.tile([128, 2, NKT, DPC], F8, tag="wq")
        wk_sb = consts.tile([128, 2, NKT, DPC], F8, tag="wk")
        wv_sb = consts.tile([128, 2, NKT, DPC], F8, tag="wv")
        wo_sb = consts.tile([128, D], BF16, tag="wo")
        bq_sb = consts.tile([128, 1], F32, tag="bq")
        bk_sb = consts.tile([128, 1], F32, tag="bk")
        bv_sb = consts.tile([128, 1], F32, tag="bv")
        ident_sb = consts.tile([128, 128], BF16, tag="ident")
        zw_sb = consts.tile([128, 512], BF16, tag="zw")
        nc.vector.memset(zw_sb, 0.0)
        sixt_sb = consts.tile([128, 1], F32, tag="sixt")
        nc.vector.memset(sixt_sb, 1.0 / 16.0)
        zero_sb = consts.tile([128, 1], F32, tag="zero")
        nc.vector.memset(zero_sb, 0.0)

        # Everything on the SP queue: HWDGE is one serial device, so a single
        # queue gives exact arrival ordering (w -> qt h0 pairs -> biases ->
        # the rest).
        def load_consts_head():
            _wpair(wk_d, wk_sb, nc.sync)
            _wpair(wq_d, wq_sb, nc.sync)

        def load_consts_rest():
            nc.sync.dma_start(out=bk_sb, in_=bk_d[:, :])
            nc.sync.dma_start(out=bq_sb, in_=bq_d[:, :])
            nc.sync.dma_start(out=bv_sb, in_=bv_d[:, :])
            _wpair(wv_d, wv_sb, nc.scalar)
            nc.scalar.dma_start(out=wo_sb, in_=wo_d[:, :])
            nc.scalar.dma_start(out=ident_sb, in_=ident_d[:, :])

        state = {}

        # ------------------- work queue machinery ---------------------------
        FWQ = deque()          # groups: [items_deque, needs]
        PROVIDED = set()
        pending = deque()      # (thunk, needs_marker_or_None)
        ACTIVE = [None]
        cur_items = [None]

        def fw(fn, provides=None):
            assert cur_items[0] is not None, "fw() outside a group"
            cur_items[0].append((fn, provides))

        def group(needs=None):
            from contextlib import contextmanager

            @contextmanager
            def _cm():
                items = deque()
                FWQ.append([items, needs])
                prev = cur_items[0]
                cur_items[0] = items
                try:
                    yield
                finally:
                    cur_items[0] = prev
            return _cm()

        def _run_item(g):
            fn, prov = g[0].popleft()
            fn()
            if prov is not None:
                if isinstance(prov, list):
                    PROVIDED.update(prov)
                else:
                    PROVIDED.add(prov)
            if not g[0]:
                if ACTIVE[0] is g:
                    ACTIVE[0] = None
                if g in FWQ:
                    FWQ.remove(g)

        def feed_one():
            g = ACTIVE[0]
            if g is not None:
                if g[1] is None or g[1] in PROVIDED:
                    _run_item(g)
                    return True
                return False
            for i, cand in enumerate(FWQ):
                if i >= 16:
                    break
                if cand[1] is None or cand[1] in PROVIDED:
                    ACTIVE[0] = cand
                    _run_item(cand)
                    return True
            return False

        def feed(n):
            for _ in range(n):
                if not feed_one():
                    return

        def feed_until(marker):
            spins = 0
            while marker not in PROVIDED:
                if not feed_one():
                    try_drain(4)
                    spins += 1
                    assert spins < 2000, f"feed_until({marker}) stuck"

        def try_drain(n):
            done = 0
            while pending and done < n:
                fn, needs = pending[0]
                if needs is not None and needs not in PROVIDED:
                    return
                pending.popleft()
                fn()
                done += 1

        def drain_all():
            while pending:
                fn, needs = pending[0]
                if needs is not None and needs not in PROVIDED:
                    feed_until(needs)
                pending.popleft()
                fn()

        # ------------------------- loads ------------------------------------
        # qtc SBUF layout: [128, 2(term), NKT, S] fp8. One DMA per
        # (kt-pair, column-half) moves both terms: pair j lands complete so
        # DoubleRow k-steps can stream behind the transfers.
        def _qt_dma(b, qt_sb, t, k0, nk, h, eng):
            """One 3D transfer: term t, kt rows [k0, k0+nk), column-half h."""
            src = qtc_d[:, :, :]
            eng.dma_start(
                out=qt_sb[:, t, k0:k0 + nk, h * 1024:(h + 1) * 1024],
                in_=bass.AP(
                    tensor=src.tensor,
                    offset=src.offset + t * D * BS + (k0 * 128) * BS
                    + b * S + h * 1024,
                    ap=[[BS, 128], [128 * BS, nk], [1, 1024]]))

        def load_qt_stream_h0(b):
            """b0 column-half 0, pair-by-pair (both terms): the warmup
            projections stream behind the transfers."""
            qt_sb = qt_pool.tile([128, 2, NKT, S], F8, tag="qt")
            state[b, "qt"] = qt_sb
            for j in range(NKT // 2):
                for t in range(2):
                    _qt_dma(b, qt_sb, t, 2 * j, 2, 0, nc.sync)

        def load_qt_h1(b):
            qt_sb = state[b, "qt"]
            for t in range(2):
                for kp in range(2):
                    _qt_dma(b, qt_sb, t, 4 * kp, 4, 1, nc.sync)

        def load_qt_bulk(b, eng):
            qt_sb = qt_pool.tile([128, 2, NKT, S], F8, tag="qt")
            state[b, "qt"] = qt_sb
            for h in range(2):
                for t in range(2):
                    for kp in range(2):
                        _qt_dma(b, qt_sb, t, 4 * kp, 4, h, eng)

        # ------------------------- projections ------------------------------
        def alloc_proj(b):
            state[b, "QT"] = projp.tile([128, S], BF16, tag="QT", name="QT")
            state[b, "KT"] = projp.tile([128, S], BF16, tag="KT", name="KT")
            state[b, "VT"] = vtp.tile([128, S], BF16, tag="VT", name="VT")

        def alloc_v(b):
            V = vpool.tile([128, NST, 2, DH + 1], BF16, tag="V", name="V")
            nc.vector.memset(V[:, :, :, DH:DH + 1], 1.0)
            state[b, "V"] = V

        DR = mybir.MatmulPerfMode.DoubleRow

        def proj_mm_j(ps, b, which, pc, j):
            """kt-pair j of a projection chunk: 3 DoubleRow terms x 2 halves."""
            _mark(nc, f"proj_mm[{b}]{which}{pc}")
            qt_sb = state[b, "qt"]
            w_sb = {"q": wq_sb, "k": wk_sb, "v": wv_sb}[which]
            sl = slice(2 * j, 2 * j + 2)
            for hh in range(2):
                cs = slice(pc * 1024 + hh * 512, pc * 1024 + (hh + 1) * 512)
                for t, (wt, xt) in enumerate(
                        ((0, 0), (1, 0), (0, 1))):  # (w8,x8),(wr8,x8),(w8,xr8)
                    nc.tensor.matmul(
                        ps[:, hh * 512:(hh + 1) * 512],
                        w_sb[:, wt, sl, :], qt_sb[:, xt, sl, cs],
                        start=(j == 0 and t == 0), stop=(j == 3 and t == 2),
                        perf_mode=DR, skip_group_check=True)

        def proj_evac(ps, b, which, pc):
            _mark(nc, f"proj_ev[{b}]{which}{pc}")
            w_b = {"q": bq_sb, "k": bk_sb, "v": bv_sb}[which]
            dst = state[b, {"q": "QT", "k": "KT", "v": "VT"}[which]]
            with tc.high_priority(offset=12):
                nc.vector.tensor_scalar(
                    out=dst[:, pc * 1024:(pc + 1) * 1024], in0=ps,
                    scalar1=sixt_sb, scalar2=w_b, op0=Alu.mult, op1=Alu.add)

        def fw_proj_chunk(b, which, pc, provides=None):
            holder = {}

            def mm(j):
                if "ps" not in holder:
                    holder["ps"] = ps_tile([128, 1024], "P")
                proj_mm_j(holder["ps"], b, which, pc, j)

            with group():
                for j in range(NKT // 2):
                    fw(lambda j=j: mm(j))
                fw(lambda: proj_evac(holder["ps"], b, which, pc),
                   provides=provides)

        # ------------------------- V transpose ------------------------------
        def tr_quad(ps, b, st0):
            _mark(nc, f"tr[{b}]")
            VT = state[b, "VT"]
            psb = ps.bitcast(BF16)
            for i in range(4):
                nc.tensor.transpose(
                    psb[:, (st0 % 8 + i) * 128:(st0 % 8 + i + 1) * 128],
                    VT[:, (st0 + i) * 128:(st0 + i + 1) * 128], ident_sb)

        def tr_copy8(ps, b, st0):
            _mark(nc, f"trc[{b}]")
            V = state[b, "V"]
            dst = V[:, st0:st0 + 8, :, 0:DH]
            psb = ps.bitcast(BF16)
            nc.vector.tensor_copy(dst, _free_reshape(psb[:, 0:1024], (8, 2, DH)))

        def fw_tr_group(b, st0):
            holder = {}

            def quad(st):
                if "ps" not in holder:
                    holder["ps"] = ps_tile([128, 1024], "P")
                tr_quad(holder["ps"], b, st)

            def cpy():
                tr_copy8(holder["ps"], b, st0)

            with group():
                fw(lambda: quad(st0))
                fw(lambda: quad(st0 + 4))
                fw(cpy, provides=("trg", b, st0))

        # ------------------------- attention --------------------------------
        def alloc_attn(b):
            state[b, "ctxT"] = ctxp.tile([128, S], BF16, tag="ctxT", name="ctxT")

        PASS_ORDER = [(0, 0, 0), (0, 0, 1), (0, 1, 0), (0, 1, 1),
                      (1, 0, 0), (1, 0, 1), (1, 1, 0), (1, 1, 1)]

        def attention_pass(b, qc, u):
            QT, KT, V = state[b, "QT"], state[b, "KT"], state[b, "V"]
            tags = ("sA", "sB")
            pss = {}
            holder = {}
            pidx = PASS_ORDER.index((b, qc, u))

            def alloc_ctx():
                _mark(nc, f"zctx[{b}]{qc}{u}")
                holder["c"] = psp.tile([128, QB, 128], F32, tag="ctx",
                                       name="ps_ctx")

            def scores(sk):
                _mark(nc, f"scores[{b}]{qc}{u}")
                ps = ps_tile([128, 1024], tags[sk % 2])
                pss[sk] = ps
                with tc.high_priority(offset=PRIO_SCORES):
                    for hh in range(2):
                        nc.tensor.matmul(
                            ps[:, hh * 512:(hh + 1) * 512],
                            KT[u * DH:(u + 1) * DH, sk * 128:(sk + 1) * 128],
                            QT[u * DH:(u + 1) * DH,
                               qc * 1024 + hh * 512:qc * 1024 + (hh + 1) * 512],
                            start=True, stop=True)

            def ctx_mms(sk, e):
                _mark(nc, f"ctx[{b}]{qc}{u}")
                cps = holder["c"]
                for qb in range(QB):
                    # bank leaders (qb 0 and 4) open the accumulation group at
                    # sk 0: start marks the whole 2KB bank pending-zero, so
                    # the other qb sub-ranges zero-init on first touch.
                    nc.tensor.matmul(
                        cps[:, qb, 0:DH + 1],
                        e[:, qb * 128:(qb + 1) * 128],
                        V[:, sk, u, :],
                        start=(sk == 0 and qb in (0, 4)),
                        stop=(sk == NST - 1),
                        skip_group_check=True)

            def norm_chain():
                _mark(nc, f"norm[{b}]{qc}{u}")
                cps = holder["c"]
                rep = normp.tile([128, QB], F32, tag="rep", name="rep")
                ctxn = normp.tile([128, QB, DH], BF16, tag="ctxn", name="ctxn")
                with tc.high_priority(offset=PRIO_NORM):
                    nc.vector.reciprocal(rep, cps[:, :, DH:DH + 1])
                    nc.vector.tensor_tensor(
                        out=ctxn, in0=cps[:, :, 0:DH],
                        in1=_bcast_free(rep[:, :], DH), op=Alu.mult)
                state[b, qc, u, "ctxn"] = ctxn
                PROVIDED.add(("ctxn", b, qc, u))

            if qc == 1:
                feed_until(("q1", b))
            pending.append((alloc_ctx, None))
            scores(0)
            for sk in range(NST):
                ps = pss.pop(sk)
                _mark(nc, f"exp[{b}]{qc}{u}")
                # e-tile rotation safety: tile buffers recycle after `bufs`
                # allocations; readers (deferred ctx matmuls) must be EMITTED
                # before the buffer is reused. Force-advance when backlogged.
                spins = 0
                while len(pending) >= 14:
                    h = pending[0][1]
                    if h is not None and h not in PROVIDED:
                        feed_until(h)
                    try_drain(8)
                    spins += 1
                    assert spins < 200, "e backlog drain stuck"
                e = expp.tile([128, 1024], BF16, tag="exp", name="exp_t")
                nc.scalar.activation(e, ps, Act.Exp, bias=zero_sb, scale=1.0)
                pending.append(
                    (lambda sk=sk, e=e: ctx_mms(sk, e),
                     ("trg", b, 0 if sk < 8 else 8)))
                if sk + 1 < NST:
                    if sk + 1 == 8 and qc == 0:
                        feed_until(("k1", b))
                    scores(sk + 1)
                feed(FEED_PER_ITER)
                try_drain(DRAIN_PER_ITER)
            # normp (rep/ctxn) rotation safety: pass N's ctxn buffer is
            # reused at pass N+2 — its readers (the ctx-transpose group of
            # pass N) must be emitted first.
            norm_needs = ("trc",) + PASS_ORDER[pidx - 2] if pidx >= 2 else None
            pending.append((norm_chain, norm_needs))

        # ---------------- ctx transpose (PSUM -> ctxT) ----------------------
        def fw_tr_ctx(b, qc, u):
            holder = {}

            def quads(j):
                _mark(nc, f"ctr[{b}]{qc}{u}")
                if j == 0:
                    holder["ps"] = ps_tile([128, 1024], "P")
                ctxn = state[b, qc, u, "ctxn"]
                psb = holder["ps"].bitcast(BF16)
                for qb in range(4 * j, 4 * j + 4):
                    nc.tensor.transpose(
                        psb[u * DH:(u + 1) * DH, qb * 128:(qb + 1) * 128],
                        ctxn[:, qb, :], ident_sb)

            def ev2():
                _mark(nc, f"cev[{b}]{qc}{u}")
                psb = holder["ps"].bitcast(BF16)
                ctxT = state[b, "ctxT"]
                with tc.high_priority(offset=8):
                    nc.vector.tensor_copy(
                        ctxT[u * DH:(u + 1) * DH, qc * 1024:(qc + 1) * 1024],
                        psb[u * DH:(u + 1) * DH, 0:1024])

            provs = [("trc", b, qc, u)]
            if u == 1:
                provs.append(("ctxT", b, qc))
            with group(needs=("ctxn", b, qc, u)):
                fw(lambda: quads(0))
                fw(lambda: quads(1))
                fw(ev2, provides=provs)

        # ------------------------- out projection ---------------------------
        def outproj_mm(ps, b, st):
            _mark(nc, f"op_mm[{b}]")
            ctxT = state[b, "ctxT"]
            with tc.high_priority(offset=PRIO_OP):
                for oc in range(2):
                    nc.tensor.matmul(ps[:, oc * 512:(oc + 1) * 512],
                                     ctxT[:, st * 128:(st + 1) * 128],
                                     wo_sb[:, oc * 512:(oc + 1) * 512],
                                     start=True, stop=True)

        def outproj_evac(ps, o2, j):
            _mark(nc, "op_ev")
            nc.vector.tensor_copy(o2[:, j, :], ps)

        def outproj_store(o2, b, st0):
            _mark(nc, "op_st")
            dst = out_d[b * S + st0 * 128: b * S + (st0 + 2) * 128, :]
            nc.sync.dma_start(
                out=bass.AP(tensor=dst.tensor, offset=dst.offset,
                            ap=[[D, 128], [128 * D, 2], [1, D]]),
                in_=o2)

        def fw_outproj(b, sts):
            sts = list(sts)
            assert len(sts) % 2 == 0
            holder = {}

            def mm(st):
                holder["ps"] = ps_tile([128, 1024], "P")
                outproj_mm(holder["ps"], b, st)

            def ev(st, j):
                if j == 0:
                    holder["o2"] = outp.tile([128, 2, D], F16, tag="o",
                                             name="o2")
                outproj_evac(holder["ps"], holder["o2"], j)

            def stre(st0):
                outproj_store(holder["o2"], b, st0)

            for i, st in enumerate(sts):
                with group(needs=("ctxT", b, st // 8)):
                    fw(lambda st=st: mm(st))
                    fw(lambda st=st, j=i % 2: ev(st, j))
                    if i % 2 == 1:
                        fw(lambda st0=sts[i - 1]: stre(st0))

        # =========================== schedule ===============================
        load_consts_head()
        load_qt_stream_h0(0)
        load_consts_rest()
        load_qt_h1(0)
        alloc_proj(0)
        alloc_v(0)
        load_qt_bulk(1, nc.sync)
        # PE p-state ramp burn: the clock needs ~3us of continuous execution
        # to reach 2.4GHz; spend it on dummy matmuls while qt streams in so
        # the real warmup projections run at full speed.
        ramp_ps = psp.tile([128, QB, 128], F32, tag="ctx", name="ps_ramp")
        rflat = bass.AP(tensor=ramp_ps.tensor, offset=ramp_ps.offset,
                        ap=[list(ramp_ps.ap[0]), [1, 1024]])
        for i in range(60):
            nc.tensor.matmul(rflat[:, 0:64], zw_sb[:, 0:128], zw_sb[:, 0:64],
                             start=True, stop=True, skip_group_check=True)
        # k0/q0/v0 interleaved on the score tags + P: all consume the same qt
        # pairs as they stream in; evacs run on ACT (idle pre-attention) + DVE.
        # v0 + its transposes complete inline so V(st 0-7) is ready before
        # attention(0,0,0)'s first ctx matmul.
        psK = ps_tile([128, 1024], "sA")
        psQ = ps_tile([128, 1024], "sB")
        psV = ps_tile([128, 1024], "P")
        for j in range(NKT // 2):
            proj_mm_j(psK, 0, "k", 0, j)
            proj_mm_j(psQ, 0, "q", 0, j)
            proj_mm_j(psV, 0, "v", 0, j)
        KT0, QT0 = state[0, "KT"], state[0, "QT"]
        nc.scalar.activation(KT0[:, 0:512], psK[:, 0:512],
                             Act.Identity, bias=bk_sb, scale=1.0 / 16.0)
        nc.vector.tensor_scalar(out=QT0[:, 0:512], in0=psQ[:, 0:512],
                                scalar1=sixt_sb, scalar2=bq_sb,
                                op0=Alu.mult, op1=Alu.add)
        nc.scalar.activation(KT0[:, 512:1024], psK[:, 512:1024],
                             Act.Identity, bias=bk_sb, scale=1.0 / 16.0)
        nc.vector.tensor_scalar(out=QT0[:, 512:1024], in0=psQ[:, 512:1024],
                                scalar1=sixt_sb, scalar2=bq_sb,
                                op0=Alu.mult, op1=Alu.add)
        proj_evac(psV, 0, "v", 0)
        psT = ps_tile([128, 1024], "P")
        tr_quad(psT, 0, 0)
        tr_quad(psT, 0, 4)
        tr_copy8(psT, 0, 0)
        PROVIDED.add(("trg", 0, 0))
        alloc_attn(0)

        # b0 leftovers weave into attention(b0); V work goes first so the
        # in-pass ctx matmuls are never gated; b1's projections overlap the
        # b0 window (qt is double-buffered) so window B's P tag is mostly
        # free for the out-projections.
        fw_proj_chunk(0, "v", 1)
        fw_tr_group(0, 8)
        fw_proj_chunk(0, "k", 1, provides=("k1", 0))
        fw_proj_chunk(0, "q", 1, provides=("q1", 0))

        def _alloc_b1():
            alloc_proj(1)
            alloc_v(1)
        with group():
            fw(_alloc_b1)
        fw_proj_chunk(1, "k", 0, provides=("k0", 1))
        fw_proj_chunk(1, "q", 0, provides=("q0", 1))

        attention_pass(0, 0, 0)
        fw_tr_ctx(0, 0, 0)
        fw_proj_chunk(1, "v", 0)
        fw_tr_group(1, 0)
        fw_proj_chunk(1, "k", 1, provides=("k1", 1))
        attention_pass(0, 0, 1)
        fw_tr_ctx(0, 0, 1)
        fw_proj_chunk(1, "v", 1)
        fw_tr_group(1, 8)
        fw_proj_chunk(1, "q", 1, provides=("q1", 1))

        attention_pass(0, 1, 0)
        fw_tr_ctx(0, 1, 0)
        fw_outproj(0, range(8))
        attention_pass(0, 1, 1)
        fw_tr_ctx(0, 1, 1)

        # ---- window B: attention(b1) + all outproj ------------------------
        feed_until(("k0", 1))
        feed_until(("q0", 1))
        alloc_attn(1)

        attention_pass(1, 0, 0)
        fw_tr_ctx(1, 0, 0)
        fw_outproj(0, range(8, NST))
        attention_pass(1, 0, 1)
        fw_tr_ctx(1, 0, 1)
        fw_outproj(1, range(8))

        attention_pass(1, 1, 0)
        fw_tr_ctx(1, 1, 0)
        attention_pass(1, 1, 1)
        fw_tr_ctx(1, 1, 1)
        drain_all()
        spins = 0
        while FWQ:
            if not feed_one():
                try_drain(4)
                spins += 1
                assert spins < 2000, "tail drain stuck"
        # pipelined tail: rotate three free 2-bank tags; each tile's evac is
        # split across DVE + ACT (both idle post-attention) to halve the
        # per-st chain latency; per-st stores
        tail_tags = ("sA", "sB", "P")
        for i, st in enumerate(range(8, NST)):
            ps = ps_tile([128, 1024], tail_tags[i % 3])
            outproj_mm(ps, 1, st)
            o_sb = outp.tile([128, 2, D], F16, tag="o", name="o2")
            nc.vector.tensor_copy(o_sb[:, 0, 0:512], ps[:, 0:512])
            nc.scalar.activation(o_sb[:, 0, 512:1024], ps[:, 512:1024],
                                 Act.Copy, bias=0.0, scale=1.0)
            nc.sync.dma_start(
                out=out_d[S + st * 128: S + (st + 1) * 128, :],
                in_=o_sb[:, 0, :])

    _split_sync_commands(nc)
    return nc


def _img(w):
    """[DPC, D] float array -> SBUF image [128, NKT*DPC] (same dtype)."""
    wt = w.T.reshape(NKT, 128, DPC).transpose(1, 0, 2)
    return np.ascontiguousarray(wt.reshape(128, NKT * DPC))


def _w_pair_img(w, sl, scale):
    """fp8 weight + residual images, stacked [2, 128, NKT*DPC].

    Values stored at 16x so the residual stays clear of e4m3 subnormals;
    the 1/16 is applied at evacuation time.
    """
    f8 = ml_dtypes.float8_e4m3
    w16 = (w[sl, :].astype(np.float32) * (16.0 * scale))
    w8 = w16.astype(f8)
    wr8 = (w16 - w8.astype(np.float32)).astype(f8)
    return np.ascontiguousarray(np.stack([_img(w8), _img(wr8)]))


def _prepare(query, q_w, q_b, k_w, k_b, v_w, v_b, out_w):
    bf = ml_dtypes.bfloat16
    f8 = ml_dtypes.float8_e4m3
    qt = np.ascontiguousarray(query.reshape(BS, D).T)  # [D, BS] f32
    qt8 = qt.astype(f8)
    qtr8 = (qt - qt8.astype(np.float32)).astype(f8)
    qtc = np.ascontiguousarray(np.stack([qt8, qtr8]))  # [2, D, BS]
    in_maps = []
    for c in range(N_CORES):
        sl = slice(c * DPC, (c + 1) * DPC)
        in_maps.append({
            "qtc": qtc,
            # softmax 1/sqrt(dh) folded into the Q weights/bias
            "wq": _w_pair_img(q_w, sl, 0.125),
            "wk": _w_pair_img(k_w, sl, 1.0),
            "wv": _w_pair_img(v_w, sl, 1.0),
            "bq": np.ascontiguousarray((q_b[sl] * 0.125).reshape(DPC, 1)),
            "bk": np.ascontiguousarray(k_b[sl].reshape(DPC, 1)),
            "bv": np.ascontiguousarray(v_b[sl].reshape(DPC, 1)),
            "wo": np.ascontiguousarray(out_w[:, sl].T).astype(bf),
        })
    return in_maps


def kernel(query, mask, q_w, q_b, k_w, k_b, v_w, v_b, out_w, out_b):
    query = np.asarray(query, dtype=np.float32)
    q_w = np.asarray(q_w, dtype=np.float32); q_b = np.asarray(q_b, dtype=np.float32)
    k_w = np.asarray(k_w, dtype=np.float32); k_b = np.asarray(k_b, dtype=np.float32)
    v_w = np.asarray(v_w, dtype=np.float32); v_b = np.asarray(v_b, dtype=np.float32)
    out_w = np.asarray(out_w, dtype=np.float32); out_b = np.asarray(out_b, dtype=np.float32)

    in_maps = _prepare(query, q_w, q_b, k_w, k_b, v_w, v_b, out_w)
    nc = _build()
    res = run_bass_kernel_spmd(nc, in_maps, core_ids=list(range(N_CORES)))
    out = np.zeros((BS, D), dtype=np.float32)
    for c in range(N_CORES):
        out += res.results[c]["out_part"]
    out += out_b[None, :]
    return out.reshape(B, S, D)
